# revision 56
# baseline (speedup 1.0000x reference)
"""Trainium2 Bass kernel for the conditioned WaveNet denoiser.

Distribution strategy (8 NeuronCores):
  - Data-parallel over batch: core b owns sample b end-to-end (block loop +
    output head), with the small weights replicated.
  - The huge stacked conditioning Dense weights Dt/Ds ([10,16,2048,128] f32,
    ~335 MB for the pair) are channel-sharded 8 ways.  Core j computes the
    conditioning planes trans[b, t, k] for ALL batches b over its 16 channels
    (a packed bf16 matmul against a host-built block-diagonal matrix of
    `condition`), quantizes them to fp8-e3m4 with a per-(plane,batch)
    power-of-2 scale, and a chunked AllToAll routes each batch's planes to
    its owner core, overlapped with the residual-block compute.
  - Consumer blocks add the fp8 planes into the conv PSUM accumulation with
    an identity matmul on the tensor engine (no DVE adds), the activation
    engine reads PSUM directly with the pow2 descale folded into its scale,
    and the residual h lives in bf16 (one DVE add per tile).
  - Producer chunks and consumer block-pairs are interleaved in the PE
    stream so the serial AllToAll chain overlaps block compute.

kernel() accepts the FULL inputs and returns the FULL [8, 2048, 1] output.
"""

import os
import sys

import numpy as np

for _p in ("/opt/trn_rl_repo",):
    if _p not in sys.path and os.path.isdir(_p):
        sys.path.insert(0, _p)

import ml_dtypes  # noqa: E402

import concourse.bass as bass  # noqa: E402
import concourse.tile as tile  # noqa: E402
from concourse import bacc, bass_utils, mybir  # noqa: E402

# Problem constants (hardcoded per the spec; kernel.py must be self-contained).
L = 10
DILATIONS = [1, 2, 4, 8, 16, 32, 64, 128, 256, 512]
T = 2048
C = 128
COND = 16
B = 8
NCORES = 8
TS = 512          # time-tile (matmul moving N / one PSUM bank of f32)
NT = T // TS      # 4 time tiles
# residual blocks per AllToAll chunk.  Each collective costs ~15us fixed +
# ~5us/MB on the serial CC stream, so few medium chunks beat many small
# ones; the first AllToAll can't start before the CC bootstrap barrier
# (~75us) anyway.
CHUNK_BLOCKS = [1, 3, 3, 3]
CHUNK_START = [0, 1, 4, 7]
NCHUNK = len(CHUNK_BLOCKS)
MAXPL = 2 * max(CHUNK_BLOCKS)

F32 = mybir.dt.float32
F32R = mybir.dt.float32r
I8 = mybir.dt.int8
F8E3 = mybir.dt.float8e3
BF16 = mybir.dt.bfloat16
BF = ml_dtypes.bfloat16
E3 = ml_dtypes.float8_e3m4

AF = mybir.ActivationFunctionType


def _tap_range(t0, n, off):
    """Valid (out_lo, length) of an out tile [t0, t0+n) for input offset off."""
    lo = max(t0, -off)
    hi = min(t0 + n, T - off)
    return lo - t0, max(0, hi - lo)


def _build_nc(has_p: bool, has_bres: bool, has_bskip: bool):
    nc = bacc.Bacc(
        "TRN2",
        target_bir_lowering=False,
        debug=False,
        num_devices=NCORES,
    )

    # ---- I/O declarations (per-core values supplied via in_maps) ----
    xw = nc.dram_tensor("xw", [1, T], BF16, kind="ExternalInput")
    wcT = nc.dram_tensor("wcT", [1, C], BF16, kind="ExternalInput")
    bcp = nc.dram_tensor("bcp", [C, 1], F32, kind="ExternalInput")
    cstat = nc.dram_tensor("cstat", [C, 64], BF16, kind="ExternalInput")
    # [lb, p, hh, t] so one plane-pair is a single contiguous-line DMA
    dtp = nc.dram_tensor("dtp", [2 * L, C, 2, T], BF16, kind="ExternalInput")
    wtp = nc.dram_tensor("wtp", [C, 6 * L, C], BF16, kind="ExternalInput")
    wsr = nc.dram_tensor("wsr", [C, 2 * L, C], BF16, kind="ExternalInput")
    w1p = nc.dram_tensor("w1p", [C, 3, 2048], BF16, kind="ExternalInput")
    b1p = nc.dram_tensor("b1p", [C, 16], F32, kind="ExternalInput")
    w2p = nc.dram_tensor("w2p", [C, 96, C], BF16, kind="ExternalInput")
    b2p = nc.dram_tensor("b2p", [C, 2], F32, kind="ExternalInput")
    w3p = nc.dram_tensor("w3p", [C, 2], BF16, kind="ExternalInput")
    b3p = nc.dram_tensor("b3p", [1, 1], F32, kind="ExternalInput")
    identp = nc.dram_tensor("identp", [C, C], I8, kind="ExternalInput")
    qscp = nc.dram_tensor("qscp", [C, 2 * L], F32, kind="ExternalInput")
    avsp = nc.dram_tensor("avsp", [C, 2 * L], F32, kind="ExternalInput")
    if has_p:
        ptp = nc.dram_tensor("ptp", [2 * L, 16, T], BF16, kind="ExternalInput")
        pstat = nc.dram_tensor("pstat", [8, 64], BF16, kind="ExternalInput")
    if has_bres:
        bresp = nc.dram_tensor("bresp", [C, L], F32, kind="ExternalInput")
    if has_bskip:
        bskips = nc.dram_tensor("bskips", [C, 1], F32, kind="ExternalInput")
    out = nc.dram_tensor("out", [1, T], F32, kind="ExternalOutput")

    rg = [list(range(NCORES))]

    with tile.TileContext(nc) as tc:
        with (
            tc.tile_pool(name="consts", bufs=1) as consts,
            tc.tile_pool(name="skipbuf", bufs=1) as skipbuf,
            tc.tile_pool(name="dram", bufs=1, space="DRAM") as dram,
        ):
            # ---- small constants (hot path first) ----
            x_sb = consts.tile([1, T], BF16)
            nc.sync.dma_start(x_sb[:], xw[:, :])
            wc_sb = consts.tile([1, C], BF16)
            nc.sync.dma_start(wc_sb[:], wcT[:, :])
            bc_sb = consts.tile([C, 1], F32)
            nc.sync.dma_start(bc_sb[:], bcp[:, :])
            cs_sb = consts.tile([C, 64], BF16)
            nc.sync.dma_start(cs_sb[:], cstat[:, :])
            id_sb = consts.tile([C, C], I8)
            nc.sync.dma_start(id_sb[:], identp[:, :])
            qsc_sb = consts.tile([C, 2 * L], F32)
            nc.sync.dma_start(qsc_sb[:], qscp[:, :])
            avs_sb = consts.tile([C, 2 * L], F32)
            nc.sync.dma_start(avs_sb[:], avsp[:, :])
            wt_sb = consts.tile([C, 6 * L, C], BF16)
            nc.gpsimd.dma_start(wt_sb[:], wtp[:, :, :])
            wsr_sb = consts.tile([C, 2 * L, C], BF16)
            nc.gpsimd.dma_start(wsr_sb[:], wsr[:, :, :])
            if has_p:
                ps_sb = consts.tile([8, 64], BF16)
                nc.sync.dma_start(ps_sb[:], pstat[:, :])
            if has_bres:
                bres_sb = consts.tile([C, L], F32)
                nc.sync.dma_start(bres_sb[:], bresp[:, :])
            if has_bskip:
                bsk_sb = consts.tile([C, 1], F32)
                nc.sync.dma_start(bsk_sb[:], bskips[:, :])

            # AllToAll bounce buffers, one pair per chunk.  Layout
            # [B, 16, nplanes, T]: per destination batch, 16 channel-rows
            # each holding all the chunk's planes contiguously, so staging
            # writes and consumer reads use nplanes*2KB DMA lines instead
            # of 2KB (the HWDGE per-line overhead dominates short lines).
            a2a_in = []
            a2a_out = []
            for c in range(NCHUNK):
                nplanes = 2 * CHUNK_BLOCKS[c]
                ain = dram.tile(
                    [B, 16, nplanes, T], I8, name=f"a2a_in{c}"
                )
                aout = dram.tile(
                    [B, 16, nplanes, T], I8, name=f"a2a_out{c}"
                )
                a2a_in.append(ain)
                a2a_out.append(aout)

            skip_sb = skipbuf.tile([C, T], F32, name="skip")
            skip_bf = skipbuf.tile([C, T], BF16, name="skipbf")

            with (
                tc.tile_pool(name="hbfbuf", bufs=2) as hbfbuf,
                tc.tile_pool(name="dtbuf", bufs=5) as dtbuf,
                tc.tile_pool(name="ptbuf", bufs=2) as ptbuf,
                tc.tile_pool(name="stgbuf", bufs=3) as stgbuf,
                tc.tile_pool(name="ttbuf", bufs=4) as ttbuf,
                tc.tile_pool(name="gbuf", bufs=2) as gbuf,
                tc.tile_pool(name="avbuf", bufs=4) as avbuf,
                tc.tile_pool(name="psum_prod", bufs=2, space="PSUM") as psum_prod,
                tc.tile_pool(name="psum_z", bufs=3, space="PSUM") as psum_z,
                tc.tile_pool(name="psum_sr", bufs=2, space="PSUM") as psum_sr,
            ):
                # ---- h = x * Wc + bc  (K=1 f32 matmul + biased bf16 cast) ----
                h_bf = hbfbuf.tile([C, T], BF16, name="hbf0")
                for it in range(NT):
                    ph = psum_z.tile([C, TS], F32, name="ph", tag="z")
                    nc.tensor.matmul(
                        ph[:],
                        wc_sb[:, :],
                        x_sb[:, bass.ts(it, TS)],
                        start=True,
                        stop=True,
                    )
                    nc.scalar.activation(
                        h_bf[:, bass.ts(it, TS)], ph[:], AF.Identity,
                        bias=bc_sb[:, 0:1],
                    )

                stgs = []

                def produce_chunk(cki):
                    npl = 2 * CHUNK_BLOCKS[cki]
                    stg = stgbuf.tile([C, MAXPL, T], F8E3, name="stg")
                    stgs.append(stg)
                    for lbc in range(npl):
                        lb = 2 * CHUNK_START[cki] + lbc
                        dt2 = dtbuf.tile([C, 2, T], BF16, name="dt2")
                        nc.sync.dma_start(dt2[:], dtp[lb])
                        if has_p:
                            # two 8-row tiles: a [8:16] slice of one [16, T]
                            # tile would put the matmul rhs at base
                            # partition 8, which the PE rejects
                            pt = []
                            for hh in range(2):
                                pth = ptbuf.tile([8, T], BF16, name=f"pt{hh}")
                                nc.sync.dma_start(
                                    pth[:], ptp[lb, 8 * hh : 8 * hh + 8, :]
                                )
                                pt.append(pth)
                        for it in range(NT):
                            ppr = psum_prod.tile([C, TS], F32, name="ppr")
                            tsl = bass.ts(it, TS)
                            for hh in range(2):
                                rows = slice(64 * hh, 64 * hh + 64)
                                last_mm[0] = nc.tensor.matmul(
                                    ppr[rows, :],
                                    cs_sb[:, :],
                                    dt2[:, hh, tsl],
                                    start=True,
                                    stop=not has_p,
                                )
                                if has_p:
                                    last_mm[0] = nc.tensor.matmul(
                                        ppr[rows, :],
                                        ps_sb[:, :],
                                        pt[hh][:, tsl],
                                        start=False,
                                        stop=True,
                                    )
                            # fp8-e3m4 quantize: pow2 scale + RNE cast on ACT
                            nc.scalar.activation(
                                stg[:, lbc, tsl], ppr[:], AF.Copy,
                                scale=qsc_sb[:, lb : lb + 1],
                            )
                def stage_and_a2a(cki, eng):
                    # staging writes (npl*2KB lines), all on the fast sync
                    # queue.  Issued AFTER every chunk's MM/quant program so
                    # a staging trigger waiting on quants never head-of-line
                    # blocks later dtp triggers in the in-order sync stream.
                    npl = 2 * CHUNK_BLOCKS[cki]
                    stg = stgs[cki]
                    for hh in range(2):
                        for bh in range(2):
                            p0 = 64 * hh + 32 * bh
                            eng.dma_start(
                                a2a_in[cki][
                                    4 * bh : 4 * bh + 4, 8 * hh : 8 * hh + 8, :, :
                                ],
                                stg[p0 : p0 + 32, 0:npl, :].bitcast(I8),
                            )
                    nc.gpsimd.collective_compute(
                        "AllToAll",
                        mybir.AluOpType.bypass,
                        replica_groups=rg,
                        ins=[a2a_in[cki][:, :, :, :].opt()],
                        outs=[a2a_out[cki][:, :, :, :].opt()],
                    )

                def consume_chunk(cki, h_bf, fence):
                    for lrel in range(CHUNK_BLOCKS[cki]):
                        l = CHUNK_START[cki] + lrel
                        d = DILATIONS[l]
                        # per-block plane load (2 planes, 4KB lines) so the
                        # first blocks of a chunk start without waiting for
                        # the whole chunk's planes
                        tt = ttbuf.tile([C, 2, T], I8, name="tt")
                        (nc.gpsimd if l % 2 == 0 else nc.scalar).dma_start(
                            tt[:],
                            a2a_out[cki][:, :, 2 * lrel : 2 * lrel + 2, :],
                        )
                        pls = (0, 1)
                        g = gbuf.tile([C, T], BF16, name="g")
                        h_bf_new = hbfbuf.tile([C, T], BF16, name="hbn")
                        for it in range(NT):
                            t0 = it * TS
                            tsl = bass.ts(it, TS)
                            acts = []
                            for br, fn in ((0, AF.Tanh), (1, AF.Sigmoid)):
                                pz = psum_z.tile([C, TS], F32, name="pz", tag="z")
                                taps = []
                                for tap, off in ((1, 0), (0, -d), (2, d)):
                                    lo, n = _tap_range(t0, TS, off)
                                    if n > 0:
                                        taps.append((tap, off, lo, n))
                                for idx, (tap, off, lo, n) in enumerate(taps):
                                    w_ap = wt_sb[:, (l * 2 + br) * 3 + tap, :]
                                    mm = nc.tensor.matmul(
                                        pz[:, lo : lo + n],
                                        w_ap,
                                        h_bf[:, t0 + lo + off : t0 + lo + off + n],
                                        start=idx == 0,
                                        stop=False,
                                    )
                                    if fence is not None:
                                        tile.add_dep_helper(
                                            mm.ins,
                                            fence.ins,
                                            reason="consumer after producer",
                                        )
                                        fence = None
                                # conditioning plane added on the PE:
                                # identity matmul over the fp8 plane
                                nc.tensor.matmul(
                                    pz[:],
                                    id_sb[:, :].bitcast(F8E3),
                                    tt[:, pls[br], tsl].bitcast(F8E3),
                                    start=False,
                                    stop=True,
                                )
                                av = avbuf.tile([C, TS], BF16, name="av")
                                nc.scalar.activation(
                                    av[:], pz[:], fn,
                                    scale=avs_sb[:, l * 2 + br : l * 2 + br + 1],
                                )
                                acts.append(av)
                            nc.vector.tensor_mul(
                                g[:, tsl], acts[0][:], acts[1][:]
                            )
                            # skip 1x1 conv, accumulated in SBUF
                            psk = psum_sr.tile([C, TS], F32, name="psk", tag="sr")
                            nc.tensor.matmul(
                                psk[:],
                                wsr_sb[:, 2 * l, :],
                                g[:, tsl],
                                start=True,
                                stop=True,
                            )
                            if l == 0:
                                nc.vector.tensor_copy(skip_sb[:, tsl], psk[:])
                            else:
                                nc.vector.tensor_add(
                                    skip_sb[:, tsl], skip_sb[:, tsl], psk[:]
                                )
                            # residual 1x1 conv + h (bf16 master)
                            prs = psum_sr.tile([C, TS], F32, name="prs", tag="sr")
                            nc.tensor.matmul(
                                prs[:],
                                wsr_sb[:, 2 * l + 1, :],
                                g[:, tsl],
                                start=True,
                                stop=True,
                            )
                            nc.vector.tensor_add(
                                h_bf_new[:, tsl], prs[:], h_bf[:, tsl]
                            )
                            if has_bres:
                                nc.scalar.activation(
                                    h_bf_new[:, tsl],
                                    h_bf_new[:, tsl],
                                    AF.Identity,
                                    bias=bres_sb[:, l : l + 1],
                                )
                        h_bf = h_bf_new
                    return h_bf

                # all producer chunks first (their AllToAll triggers fire as
                # the serial CC stream drains), then the consumer blocks.
                # The fence keeps the in-order PE stream strictly
                # producer-first: a consumer matmul scheduled between
                # producer matmuls would head-of-line-block them while
                # waiting for its AllToAll.
                # chunk 0 stages early via the otherwise-idle gpsimd queue
                # (a waiting staging trigger there blocks nothing); later
                # chunks stage on the fast sync queue after ALL dtp triggers
                # so the in-order sync stream never stalls behind quants.
                last_mm = [None]
                produce_chunk(0)
                stage_and_a2a(0, nc.gpsimd)
                for cki in range(1, NCHUNK):
                    produce_chunk(cki)
                for cki in range(1, NCHUNK):
                    stage_and_a2a(cki, nc.sync)
                fence = last_mm[0]
                for cki in range(NCHUNK):
                    h_bf = consume_chunk(cki, h_bf, fence)
                    fence = None

                for it in range(NT):
                    tsl = bass.ts(it, TS)
                    if has_bskip:
                        nc.scalar.activation(
                            skip_bf[:, tsl], skip_sb[:, tsl], AF.Identity,
                            bias=bsk_sb[:, 0:1],
                        )
                    else:
                        nc.scalar.activation(
                            skip_bf[:, tsl], skip_sb[:, tsl], AF.Copy
                        )

            # ---- output head ----
            with (
                tc.tile_pool(name="o1buf", bufs=1) as o1buf,
                tc.tile_pool(name="o2buf", bufs=1) as o2buf,
                tc.tile_pool(name="obuf", bufs=1) as obuf,
                tc.tile_pool(name="psum_h", bufs=6, space="PSUM") as psum_h,
                tc.tile_pool(name="psum_h3", bufs=1, space="PSUM") as psum_h3,
                tc.tile_pool(name="headw", bufs=1) as headw,
            ):
                w1_sb = headw.tile([C, 3, 2048], BF16)
                nc.sync.dma_start(w1_sb[:], w1p[:, :, :])
                b1_sb = headw.tile([C, 16], F32)
                nc.sync.dma_start(b1_sb[:], b1p[:, :])
                w2_sb = headw.tile([C, 96, C], BF16)
                nc.sync.dma_start(w2_sb[:], w2p[:, :, :])
                b2_sb = headw.tile([C, 2], F32)
                nc.sync.dma_start(b2_sb[:], b2p[:, :])
                w3_sb = headw.tile([C, 2], BF16)
                nc.sync.dma_start(w3_sb[:], w3p[:, :])
                b3_sb = headw.tile([1, 1], F32)
                nc.sync.dma_start(b3_sb[:], b3p[:, :])

                out1 = o1buf.tile([C, 16, T], BF16, name="out1")
                out2 = o2buf.tile([C, 2, T], BF16, name="out2")
                o_sb = obuf.tile([1, T], F32, name="o_sb")
                # W1: one LDWEIGHTS per (oc, tap), accumulated across the 4
                # time-tile PSUM banks (weight loads amortized 4x)
                for oc in range(16):
                    p1s = [
                        psum_h.tile([C, TS], F32, name="p1", tag="ph")
                        for _ in range(NT)
                    ]
                    for ti, (tap, off) in enumerate(((1, 0), (0, -1), (2, 1))):
                        w_ap = w1_sb[:, tap, oc * C : (oc + 1) * C]
                        for it in range(NT):
                            t0 = it * TS
                            lo, n = _tap_range(t0, TS, off)
                            if n == 0:
                                continue
                            nc.tensor.matmul(
                                p1s[it][:, lo : lo + n],
                                w_ap,
                                skip_bf[:, t0 + lo + off : t0 + lo + off + n],
                                start=ti == 0,
                                stop=ti == 2,
                            )
                    for it in range(NT):
                        nc.scalar.activation(
                            out1[:, oc, bass.ts(it, TS)],
                            p1s[it][:],
                            AF.Relu,
                            bias=b1_sb[:, oc : oc + 1],
                        )
                # W2: one LDWEIGHTS per (oc2, tap, ic), accumulated across
                # the 4 time-tile PSUM banks
                for oc2 in range(2):
                    p2s = [
                        psum_h.tile([C, TS], F32, name="p2", tag="ph")
                        for _ in range(NT)
                    ]
                    nw = 0
                    for tap, off in ((1, 0), (0, -1), (2, 1)):
                        for ic in range(16):
                            w_ap = w2_sb[:, (tap * 16 + ic) * 2 + oc2, :]
                            for it in range(NT):
                                t0 = it * TS
                                lo, n = _tap_range(t0, TS, off)
                                if n == 0:
                                    continue
                                nc.tensor.matmul(
                                    p2s[it][:, lo : lo + n],
                                    w_ap,
                                    out1[:, ic, t0 + lo + off : t0 + lo + off + n],
                                    start=nw == 0,
                                    stop=nw == 47,
                                )
                            nw += 1
                    for it in range(NT):
                        nc.scalar.activation(
                            out2[:, oc2, bass.ts(it, TS)],
                            p2s[it][:],
                            AF.Relu,
                            bias=b2_sb[:, oc2 : oc2 + 1],
                        )
                for it in range(NT):
                    tsl = bass.ts(it, TS)
                    p3 = psum_h3.tile([1, TS], F32, name="p3")
                    for ic in range(2):
                        nc.tensor.matmul(
                            p3[:],
                            w3_sb[:, ic : ic + 1],
                            out2[:, ic, tsl],
                            start=ic == 0,
                            stop=ic == 1,
                        )
                    nc.scalar.activation(
                        o_sb[:, tsl], p3[:], AF.Tanh, bias=b3_sb[:, 0:1]
                    )
                nc.sync.dma_start(out[:, :], o_sb[:])

    nc.compile()
    return nc


_NC_CACHE = {}


def _get_nc(has_p, has_bres, has_bskip):
    key = (has_p, has_bres, has_bskip)
    if key not in _NC_CACHE:
        _NC_CACHE[key] = _build_nc(*key)
    return _NC_CACHE[key]


def _pack_inputs(
    x, condition, Wc, bc, Wt, bt, Ws, bs, Dt, Bt, Ds, Bs,
    Wskip, bskip, Wres, bres, W1, b1, W2, b2, W3, b3,
):
    """Host-side sharding + layout packs. Returns (in_maps, flags)."""
    f32 = np.float32
    x = np.asarray(x, f32)
    condition = np.asarray(condition, f32)
    has_p = bool(
        np.any(np.asarray(Bt)) or np.any(np.asarray(Bs))
        or np.any(np.asarray(bt)) or np.any(np.asarray(bs))
    )
    has_bres = bool(np.any(np.asarray(bres)))
    has_bskip = bool(np.any(np.asarray(bskip)))

    # fp8-e3m4 calibration: per-(l,branch,batch) power-of-2 scale placing
    # the exact plane amax in (4, 8] (e3m4 max normal is 15.5; the ~2x
    # margin covers device bf16 rounding).  Scales are runtime data.
    Dt_ = np.asarray(Dt, f32)
    Ds_ = np.asarray(Ds, f32)
    s2 = np.ones((L, 2, B), f32)  # s2[l, br, b], power of 2
    for l in range(L):
        for br, Dn in ((0, Dt_), (1, Ds_)):
            M = condition @ Dn[l].reshape(COND, T * C)  # [B, T*C]
            amax = np.abs(M).max(axis=1)
            if has_p:
                Pn = (np.asarray(Bt if br == 0 else Bs, f32)[l]
                      + np.asarray(bt if br == 0 else bs, f32)[l][None, :])
                amax = amax + float(np.abs(Pn).max())
            amax = np.maximum(amax, 1e-6)
            s2[l, br] = 2.0 ** np.floor(np.log2(8.0 / amax))
    # qscp[p=64hh+8b+g, lb] = s2[lb, b]  (same on every core)
    qscp = np.zeros((C, 2 * L), f32)
    for hh in range(2):
        for b in range(B):
            for g in range(8):
                qscp[64 * hh + 8 * b + g, :] = s2[:, :, b].reshape(2 * L)
    # avsp per core b: 1/s2[lb, b] broadcast over partitions
    avsp_all = np.ascontiguousarray(
        np.broadcast_to(
            (1.0 / s2).reshape(2 * L, B).T[:, None, :], (B, C, 2 * L)
        ).copy()
    )

    # dtp: [core, lb=2l+br, hh, p=16g+c, t] = D_br[l, c, t, 16j+8hh+g]
    D = np.stack([Dt_, Ds_], axis=1)
    D = D.reshape(L, 2, COND, T, 8, 2, 8)
    # [core, lb, p=16g+c, hh, t]
    dtp_all = np.ascontiguousarray(
        D.transpose(4, 0, 1, 6, 2, 5, 3).reshape(NCORES, 2 * L, C, 2, T)
    ).astype(BF)
    del D

    # cstat: [16g+c, 8b+g] = condition[b, c]
    cstat = np.zeros((C, 64), f32)
    for g in range(8):
        cstat[16 * g : 16 * g + 16, g::8] = condition.T
    cstat = cstat.astype(BF)

    # wtp per core b: [cin, (l,br,tap), cout], pre-scaled by s2[l,br,b] so
    # the conv PSUM is in plane-quant units
    Wg = np.stack([np.asarray(Wt, f32), np.asarray(Ws, f32)], axis=1)
    wtp_all = []
    for b in range(B):
        Wgb = Wg * s2[:, :, b][:, :, None, None, None]
        wtp_all.append(np.ascontiguousarray(
            Wgb.transpose(3, 0, 1, 2, 4).reshape(C, 6 * L, C)
        ).astype(BF))
    # wsr: [cin, (l, skip/res), cout]
    Ssr = np.stack([np.asarray(Wskip, f32)[:, 0], np.asarray(Wres, f32)[:, 0]], axis=1)
    wsr = np.ascontiguousarray(Ssr.transpose(2, 0, 1, 3).reshape(C, 2 * L, C)).astype(BF)

    w1p = np.ascontiguousarray(np.asarray(W1, f32).transpose(1, 0, 2)).astype(BF)
    b1p = np.ascontiguousarray(np.asarray(b1, f32).reshape(16, C).T)
    w2p = np.ascontiguousarray(
        np.asarray(W2, f32).reshape(3, 16, C, 2, C).transpose(2, 0, 1, 3, 4)
        .reshape(C, 96, C)
    ).astype(BF)
    b2p = np.ascontiguousarray(np.asarray(b2, f32).reshape(2, C).T)
    w3p = np.ascontiguousarray(np.asarray(W3, f32)[0, :, 0].reshape(2, C).T).astype(BF)
    b3p = np.asarray(b3, f32).reshape(1, 1)
    wcT = np.ascontiguousarray(np.asarray(Wc, f32).reshape(1, C)).astype(BF)
    bcp = np.asarray(bc, f32).reshape(C, 1)
    identp = np.eye(C, dtype=f32).astype(E3).view(np.int8)

    base = {
        "wcT": wcT, "bcp": bcp, "cstat": cstat, "wsr": wsr,
        "w1p": w1p, "b1p": b1p, "w2p": w2p, "b2p": b2p, "w3p": w3p,
        "b3p": b3p, "identp": identp, "qscp": qscp,
    }
    if has_p:
        P = np.stack(
            [
                np.asarray(Bt, f32) + np.asarray(bt, f32)[:, None, :],
                np.asarray(Bs, f32) + np.asarray(bs, f32)[:, None, :],
            ],
            axis=1,
        )  # [L, 2, T, C]
        P = P.reshape(L, 2, T, 8, 2, 8)
        ptp_all = np.ascontiguousarray(
            P.transpose(3, 0, 1, 4, 5, 2).reshape(NCORES, 2 * L, 16, T)
        ).astype(BF)
        del P
        pstat = np.zeros((8, 64), f32)
        for g in range(8):
            pstat[g, g::8] = 1.0
        base["pstat"] = pstat.astype(BF)
    if has_bres:
        base["bresp"] = np.ascontiguousarray(np.asarray(bres, f32).T)
    if has_bskip:
        base["bskips"] = np.asarray(bskip, f32).sum(axis=0).reshape(C, 1)

    in_maps = []
    for j in range(NCORES):
        m = dict(base)
        m["xw"] = np.ascontiguousarray(x[j, :, 0].reshape(1, T)).astype(BF)
        m["dtp"] = dtp_all[j]
        m["wtp"] = wtp_all[j]
        m["avsp"] = avsp_all[j]
        if has_p:
            m["ptp"] = ptp_all[j]
        in_maps.append(m)
    return in_maps, (has_p, has_bres, has_bskip)


def kernel(**inputs) -> np.ndarray:
    in_maps, flags = _pack_inputs(**inputs)
    nc = _get_nc(*flags)
    res = bass_utils.run_bass_kernel_spmd(
        nc, in_maps, core_ids=list(range(NCORES))
    )
    outs = [res.results[j]["out"].reshape(T, 1) for j in range(NCORES)]
    return np.stack(outs, axis=0).astype(np.float32)
